# revision 11
# baseline (speedup 1.0000x reference)
"""Trainium2 Bass kernel for nn_CrossModelAttention (gnn_message_passing).

Distribution (8 NeuronCores, one SPMD NEFF):
  - lm head + LayerNorm: node-sharded (256 nodes/core), AllGather -> full lm^T
  - RGCN x2: relation GEMMs replicated (cheap), edge aggregation sharded by
    dst node (edges sorted by dst on host, per-core index/scale arrays);
    segment-sum done on the PE via per-tile one-hot selection matmuls;
    messages gathered with dma_gather from a per-core fp16 xr table.
    AllGather -> full g^T after each layer.
  - Attention: query-sharded (each core does all 8 heads for its 256 query
    nodes); no collective needed for the head merge.
  - Residual + BatchNorm: channel stats partial-summed locally, 1KB
    AllReduce, normalization + classifier local; per-core [256, 8] outputs
    concatenated on host.

Layouts: activations kept feature-major ("T layout", [128 feat partitions,
nodes free]) so per-feature params are per-partition scalars and no
transposes are needed anywhere. Matmul inputs fp16, PSUM/stats fp32.
"""

import os
import sys

if "/opt/trn_rl_repo" not in sys.path:
    sys.path.insert(0, "/opt/trn_rl_repo")

import numpy as np

import concourse.bacc as bacc
import concourse.bass as bass
import concourse.mybir as mybir
import concourse.tile as tile
from concourse.bass_utils import run_bass_kernel_spmd

F32 = mybir.dt.float32
F16 = mybir.dt.float16
I16 = mybir.dt.int16
AF = mybir.ActivationFunctionType
OP = mybir.AluOpType

N = 2048          # nodes (B*S)
D = 1024          # input dim
HID = 128
NR = 3            # relations
NL = 2            # rgcn layers
NH = 8            # heads
DH = 16
NCORES = 8
NPC = N // NCORES  # nodes per core = 256
EPS = 1e-5

LAST_RESULT = None  # BassKernelResults of the most recent run (for test harness)


def _ensure_profile_hook():
    """Install the NTFF profile hook if boot() could not (antenv.axon_hooks
    may be missing from the image). Only matters when BASS_TRACE=1."""
    try:
        import antenv.axon_hooks as ah
        if ah.get_axon_ntff_profile_hook() is None:
            from trn_agent_boot.trn_boot import _ntff_profile_via_ctypes
            hook = _ntff_profile_via_ctypes("/opt/axon/libaxon_pjrt.so")
            if hook is not None:
                ah.set_axon_ntff_profile_hook(hook)
    except Exception:
        pass


def _bcast_ap(dram_ap, parts, free):
    """DMA access pattern broadcasting a [free] dram vector across partitions."""
    return bass.AP(tensor=dram_ap.tensor, offset=dram_ap.offset, ap=[[0, parts], [1, free]])


def _gather_cc_ap(cc):
    """AP over cc_out [R, 128, NPC] reading as [128 feat, R*NPC nodes]."""
    return bass.AP(tensor=cc[:].tensor, offset=0,
                   ap=[[NPC, 128], [128 * NPC, NCORES], [1, NPC]])


def build(nc, EPAD):
    NT = EPAD // 128
    groups = [list(range(NCORES))]

    # ---------------- dram tensors ----------------
    outT_d = nc.dram_tensor("outT", [D, NPC], F16, kind="ExternalInput")
    lmw_d = nc.dram_tensor("lm_w", [D, HID], F16, kind="ExternalInput")
    lmb_d = nc.dram_tensor("lm_b", [HID], F32, kind="ExternalInput")
    lng_d = nc.dram_tensor("ln_g", [HID], F32, kind="ExternalInput")
    lnb_d = nc.dram_tensor("ln_b", [HID], F32, kind="ExternalInput")
    root_d = nc.dram_tensor("root", [NL, HID, HID], F16, kind="ExternalInput")
    rel_d = nc.dram_tensor("rel", [NL, HID, NR * HID], F16, kind="ExternalInput")
    rgb_d = nc.dram_tensor("rgb", [NL, HID], F32, kind="ExternalInput")
    wq_d = nc.dram_tensor("wq", [HID, HID], F16, kind="ExternalInput")
    wk_d = nc.dram_tensor("wk", [HID, HID], F16, kind="ExternalInput")
    wv_d = nc.dram_tensor("wv", [HID, HID], F16, kind="ExternalInput")
    wop_d = nc.dram_tensor("wop", [DH + 1, NH, HID], F16, kind="ExternalInput")
    boe_d = nc.dram_tensor("boe", [HID], F32, kind="ExternalInput")
    bng_d = nc.dram_tensor("bn_g", [HID], F32, kind="ExternalInput")
    bnb_d = nc.dram_tensor("bn_b", [HID], F32, kind="ExternalInput")
    clsw_d = nc.dram_tensor("cls_w", [HID, NH], F16, kind="ExternalInput")
    clsb_d = nc.dram_tensor("cls_b", [NH], F32, kind="ExternalInput")
    adj_d = nc.dram_tensor("adj", [128, N // 128, NR, NPC], F16, kind="ExternalInput")

    y_d = nc.dram_tensor("y", [NPC, NH], F32, kind="ExternalOutput")

    cc_in = [nc.dram_tensor(f"cci{i}", [128, NPC], F16, kind="Internal") for i in range(3)]
    cc_out = [nc.dram_tensor(f"cco{i}", [NCORES, 128, NPC], F16, kind="Internal",
                             addr_space="Shared") for i in range(3)]
    bn_in = nc.dram_tensor("bni", [128, 2], F32, kind="Internal")
    bn_out = nc.dram_tensor("bno", [128, 2], F32, kind="Internal", addr_space="Shared")

    with tile.TileContext(nc) as tc:
        with tc.tile_pool(name="const", bufs=1) as cst, \
             tc.tile_pool(name="persist", bufs=1) as per, \
             tc.tile_pool(name="work", bufs=2) as wk, \
             tc.tile_pool(name="small", bufs=2) as sm, \
             tc.tile_pool(name="epool", bufs=2) as ep, \
             tc.tile_pool(name="msgp", bufs=1) as mp, \
             tc.tile_pool(name="psA", bufs=2, space="PSUM") as psA, \
             tc.tile_pool(name="psB", bufs=1, space="PSUM") as psB, \
             tc.tile_pool(name="psAcc", bufs=1, space="PSUM") as psAcc, \
             tc.tile_pool(name="psSt", bufs=3, space="PSUM") as psSt:

            # ---------------- constants to SBUF ----------------
            lmw_sb = cst.tile([128, D // 128, HID], F16)
            nc.sync.dma_start(out=lmw_sb[:], in_=lmw_d[:].rearrange("(k p) f -> p k f", p=128))
            outT_sb = cst.tile([128, D // 128, NPC], F16)
            nc.sync.dma_start(out=outT_sb[:], in_=outT_d[:].rearrange("(k p) n -> p k n", p=128))
            root_sb = cst.tile([128, NL, HID], F16)
            nc.sync.dma_start(out=root_sb[:], in_=root_d[:].rearrange("l k f -> k l f"))
            rel_sb = cst.tile([128, NL, NR * HID], F16)
            nc.sync.dma_start(out=rel_sb[:], in_=rel_d[:].rearrange("l k f -> k l f"))
            wq_sb = cst.tile([128, HID], F16)
            nc.sync.dma_start(out=wq_sb[:], in_=wq_d[:])
            wk_sb = cst.tile([128, HID], F16)
            nc.sync.dma_start(out=wk_sb[:], in_=wk_d[:])
            wv_sb = cst.tile([128, HID], F16)
            nc.sync.dma_start(out=wv_sb[:], in_=wv_d[:])
            wop_sb = cst.tile([DH + 1, NH, HID], F16)
            nc.sync.dma_start(out=wop_sb[:], in_=wop_d[:])
            cls_sb = cst.tile([128, NH], F16)
            nc.sync.dma_start(out=cls_sb[:], in_=clsw_d[:])

            def vec128(d):
                t = cst.tile([128, 1], F32, tag=f"v_{d.name}")
                nc.sync.dma_start(out=t[:], in_=d[:, None])
                return t
            lmb_sb = vec128(lmb_d)
            lng_sb = vec128(lng_d)
            lnb_sb = vec128(lnb_d)
            bng_sb = vec128(bng_d)
            bnb_sb = vec128(bnb_d)
            boe_sb = vec128(boe_d)
            rgb_sb = cst.tile([128, NL], F32)
            nc.sync.dma_start(out=rgb_sb[:], in_=rgb_d[:].rearrange("l f -> f l"))
            clsb_bc = cst.tile([128, NH], F32)
            nc.gpsimd.dma_start(out=clsb_bc[:], in_=_bcast_ap(clsb_d[:], 128, NH))
            adj_sb = cst.tile([128, N // 128, NR, NPC], F16)
            nc.sync.dma_start(out=adj_sb[:], in_=adj_d[:])

            ones_col = cst.tile([128, 1], F32)
            nc.vector.memset(ones_col[:], 1.0)
            ones_1x128 = cst.tile([1, 128], F32)
            nc.vector.memset(ones_1x128[:], 1.0)
            ones_1x17 = cst.tile([1, DH + 1], F32)
            nc.vector.memset(ones_1x17[:], 1.0)
            eps1 = cst.tile([1, 1], F32)
            nc.vector.memset(eps1[:], EPS)
            eps128 = cst.tile([128, 1], F32)
            nc.vector.memset(eps128[:], EPS)

            def ps_acc():
                return psAcc.tile([128, NPC], F32, tag="acc", name="acc")

            def ps_mm():
                return psA.tile([128, NR * HID], F32, tag="mm", name="mm")

            def ps_b():
                return psB.tile([128, 512], F32, tag="b", name="b")

            # ---------------- phase 1: lm head + LN (own nodes) ----------------
            lm_ps = ps_acc()
            for k in range(D // 128):
                nc.tensor.matmul(lm_ps[:], lhsT=lmw_sb[:, k, :], rhs=outT_sb[:, k, :],
                                 start=(k == 0), stop=(k == D // 128 - 1))
            r_sb = per.tile([128, NPC], F32, tag="r")
            nc.scalar.activation(out=r_sb[:], in_=lm_ps[:], func=AF.Relu, bias=lmb_sb[:], scale=1.0)
            sq = wk.tile([128, NPC], F32, tag="sq")
            nc.vector.tensor_mul(sq[:], r_sb[:], r_sb[:])
            st_ps = ps_b()[0:1, :]
            nc.tensor.matmul(st_ps[:, 0:NPC], lhsT=ones_col[:], rhs=r_sb[:], start=True, stop=True)
            nc.tensor.matmul(st_ps[:, NPC:], lhsT=ones_col[:], rhs=sq[:], start=True, stop=True)
            mu_r = sm.tile([1, NPC], F32, tag="mu")
            nc.vector.tensor_scalar_mul(mu_r[:], st_ps[:, 0:NPC], 1.0 / HID)
            ex2_r = sm.tile([1, NPC], F32, tag="ex2")
            nc.vector.tensor_scalar_mul(ex2_r[:], st_ps[:, NPC:], 1.0 / HID)
            var_r = sm.tile([1, NPC], F32, tag="var")
            nc.vector.tensor_mul(var_r[:], mu_r[:], mu_r[:])
            nc.vector.tensor_sub(var_r[:], ex2_r[:], var_r[:])
            sd_r = sm.tile([1, NPC], F32, tag="sd")
            nc.scalar.activation(out=sd_r[:], in_=var_r[:], func=AF.Sqrt, bias=eps1[:], scale=1.0)
            pk = sm.tile([1, 2 * NPC], F32, tag="pk")
            nc.vector.reciprocal(pk[:, 0:NPC], sd_r[:])
            nc.vector.tensor_mul(pk[:, NPC:], mu_r[:], pk[:, 0:NPC])
            bc_ps = ps_b()
            nc.tensor.matmul(bc_ps[:], lhsT=ones_1x128[:], rhs=pk[:], start=True, stop=True)
            lmT_own = per.tile([128, NPC], F32, tag="lmT_own")
            nc.vector.tensor_mul(lmT_own[:], r_sb[:], bc_ps[:, 0:NPC])
            nc.vector.tensor_sub(lmT_own[:], lmT_own[:], bc_ps[:, NPC:])
            nc.vector.tensor_scalar(out=lmT_own[:], in0=lmT_own[:], scalar1=lng_sb[:],
                                    scalar2=lnb_sb[:], op0=OP.mult, op1=OP.add)
            lm16_own = per.tile([128, NPC], F16, tag="lm16_own")
            nc.vector.tensor_copy(lm16_own[:], lmT_own[:])

            q16_all = per.tile([DH, NH, NPC], F16, tag="q16a")
            for h in range(NH):
                q_ps = ps_b()[0:DH, 0:NPC]
                nc.tensor.matmul(q_ps[:], lhsT=wq_sb[:, DH * h:DH * (h + 1)], rhs=lm16_own[:],
                                 start=True, stop=True)
                nc.vector.tensor_copy(q16_all[:, h, :], q_ps[:])

            nc.sync.dma_start(out=cc_in[0][:], in_=lm16_own[:])
            nc.gpsimd.collective_compute(
                kind="AllGather", op=OP.bypass, replica_groups=groups,
                ins=[cc_in[0][:]], outs=[cc_out[0][:]])
            lmT_full = per.tile([128, N], F16, tag="lmT_full")
            nc.sync.dma_start(out=lmT_full[:].rearrange("f (r n) -> f r n", r=NCORES), in_=_gather_cc_ap(cc_out[0]))

            # ---------------- phase 2: RGCN layers ----------------
            xT = lmT_full
            x16_own = lm16_own
            g16_own = None
            for l in range(NL):
                xr_all = mp.tile([128, N // 128, NR, HID], F16, tag="xr_all", name="xr_all")
                for c in range(N // 128):
                    xr_ps = ps_mm()
                    nc.tensor.matmul(xr_ps[:], lhsT=xT[:, 128 * c:128 * (c + 1)],
                                     rhs=rel_sb[:, l, :], start=True, stop=True)
                    nc.vector.tensor_copy(
                        xr_all[:, c, :, :],
                        xr_ps[:].rearrange("p (r f) -> p r f", r=NR))
                agg_ps = ps_acc()
                nc.tensor.matmul(agg_ps[:], lhsT=root_sb[:, l, :], rhs=x16_own[:],
                                 start=True, stop=False)
                for c in range(N // 128):
                    for r in range(NR):
                        nc.tensor.matmul(agg_ps[:], lhsT=xr_all[:, c, r, :],
                                         rhs=adj_sb[:, c, r, :], start=False,
                                         stop=(c == N // 128 - 1 and r == NR - 1))
                g16_own = per.tile([128, NPC], F16, tag=f"g16_own{l}")
                nc.scalar.activation(out=g16_own[:], in_=agg_ps[:], func=AF.Relu,
                                     bias=rgb_sb[:, l:l + 1], scale=1.0)
                nc.sync.dma_start(out=cc_in[1 + l][:], in_=g16_own[:])
                nc.gpsimd.collective_compute(
                    kind="AllGather", op=OP.bypass, replica_groups=groups,
                    ins=[cc_in[1 + l][:]], outs=[cc_out[1 + l][:]])
                gT_full = per.tile([128, N], F16, tag=f"gT_full{l}")
                nc.sync.dma_start(out=gT_full[:].rearrange("f (r n) -> f r n", r=NCORES), in_=_gather_cc_ap(cc_out[1 + l]))
                xT = gT_full
                x16_own = g16_own

            gT = xT  # final graph features, feature-major, fp16

            # ---------------- phase 3: attention (all heads, own queries) ----------------
            vaug = per.tile([128, N // 128, NH, DH + 1], F16, tag="vaug")
            nc.vector.memset(vaug[:, :, :, 0:1], 1.0)
            for c in range(N // 128):
                v_ps = ps_mm()[:, 0:HID]
                nc.tensor.matmul(v_ps[:], lhsT=gT[:, 128 * c:128 * (c + 1)], rhs=wv_sb[:],
                                 start=True, stop=True)
                nc.vector.tensor_copy(
                    vaug[:, c, :, 1:DH + 1],
                    v_ps[:].rearrange("p (h d) -> p h d", h=NH))

            attn_ps = ps_acc()
            for h in range(NH):
                q16 = q16_all[:, h, :]
                k16 = wk.tile([DH, N], F16, tag="k16")
                for j in range(N // 512):
                    k_ps = ps_b()[0:DH, :]
                    nc.tensor.matmul(k_ps[:], lhsT=wk_sb[:, DH * h:DH * (h + 1)],
                                     rhs=gT[:, 512 * j:512 * (j + 1)], start=True, stop=True)
                    nc.vector.tensor_copy(k16[:, 512 * j:512 * (j + 1)], k_ps[:])
                num_ps = ps_b()[0:DH + 1, 0:NPC]
                e16 = ep.tile([128, N // 256, 2, NPC], F16, tag="e16", name="e16")
                for jp in range(N // 256):
                    st_ps = psSt.tile([128, 2, NPC], F32, tag="stps", name="stps")
                    for u in range(2):
                        j = 2 * jp + u
                        nc.tensor.matmul(st_ps[:, u, :], lhsT=k16[:, 128 * j:128 * (j + 1)],
                                         rhs=q16[:], start=True, stop=True)
                    nc.scalar.activation(out=e16[:, jp, :, :], in_=st_ps[:], func=AF.Exp)
                for j in range(N // 128):
                    nc.tensor.matmul(num_ps[:], lhsT=vaug[:, j, h, :],
                                     rhs=e16[:, j // 2, j % 2, :],
                                     start=(j == 0), stop=(j == N // 128 - 1))
                num_sb = sm.tile([DH + 1, NPC], F32, tag="num")
                nc.vector.tensor_copy(num_sb[:], num_ps[:])
                rden = sm.tile([1, NPC], F32, tag="rden")
                nc.vector.reciprocal(rden[:], num_sb[0:1, :])
                dbc_ps = ps_b()[0:DH + 1, 0:NPC]
                nc.tensor.matmul(dbc_ps[:], lhsT=ones_1x17[:], rhs=rden[:], start=True, stop=True)
                ctx16 = sm.tile([DH + 1, NPC], F16, tag="ctx16")
                nc.vector.tensor_mul(ctx16[:], num_sb[:], dbc_ps[:])
                nc.tensor.matmul(attn_ps[:], lhsT=wop_sb[:, h, :], rhs=ctx16[:],
                                 start=(h == 0), stop=(h == NH - 1))

            # ---------------- phase 4: residual + BN + classifier ----------------
            fused = per.tile([128, NPC], F32, tag="fused")
            nc.vector.scalar_tensor_tensor(out=fused[:], in0=attn_ps[:], scalar=boe_sb[:],
                                           in1=lmT_own[:], op0=OP.add, op1=OP.add)
            fsq = wk.tile([128, NPC], F32, tag="fsq")
            nc.vector.tensor_mul(fsq[:], fused[:], fused[:])
            bnp = sm.tile([128, 2], F32, tag="bnp")
            nc.vector.tensor_reduce(bnp[:, 0:1], fused[:], mybir.AxisListType.X, OP.add)
            nc.vector.tensor_reduce(bnp[:, 1:2], fsq[:], mybir.AxisListType.X, OP.add)
            nc.sync.dma_start(out=bn_in[:], in_=bnp[:])
            nc.gpsimd.collective_compute(
                kind="AllReduce", op=OP.add, replica_groups=groups,
                ins=[bn_in[:]], outs=[bn_out[:]])
            bnst = sm.tile([128, 2], F32, tag="bnst")
            nc.sync.dma_start(out=bnst[:], in_=bn_out[:])
            mu_c = sm.tile([128, 1], F32, tag="muc")
            nc.vector.tensor_scalar_mul(mu_c[:], bnst[:, 0:1], 1.0 / N)
            var_c = sm.tile([128, 1], F32, tag="varc")
            nc.vector.tensor_scalar_mul(var_c[:], bnst[:, 1:2], 1.0 / N)
            mu2_c = sm.tile([128, 1], F32, tag="mu2c")
            nc.vector.tensor_mul(mu2_c[:], mu_c[:], mu_c[:])
            nc.vector.tensor_sub(var_c[:], var_c[:], mu2_c[:])
            sd_c = sm.tile([128, 1], F32, tag="sdc")
            nc.scalar.activation(out=sd_c[:], in_=var_c[:], func=AF.Sqrt, bias=eps128[:], scale=1.0)
            scl_c = sm.tile([128, 1], F32, tag="sclc")
            nc.vector.reciprocal(scl_c[:], sd_c[:])
            nc.vector.tensor_mul(scl_c[:], scl_c[:], bng_sb[:])
            shf_c = sm.tile([128, 1], F32, tag="shfc")
            nc.vector.tensor_mul(shf_c[:], mu_c[:], scl_c[:])
            nc.vector.tensor_sub(shf_c[:], bnb_sb[:], shf_c[:])
            fn16 = wk.tile([128, NPC], F16, tag="fn16")
            nc.vector.tensor_scalar(out=fn16[:], in0=fused[:], scalar1=scl_c[:],
                                    scalar2=shf_c[:], op0=OP.mult, op1=OP.add)
            yv = y_d[:].rearrange("(c p) f -> c p f", p=128)
            for c in range(NPC // 128):
                lg_ps = ps_mm()[:, 0:NH]
                nc.tensor.matmul(lg_ps[:], lhsT=fn16[:, 128 * c:128 * (c + 1)], rhs=cls_sb[:],
                                 start=True, stop=True)
                out_sb = wk.tile([128, NH], F32, tag="outsb")
                nc.vector.tensor_add(out_sb[:], lg_ps[:], clsb_bc[:])
                nc.sync.dma_start(out=yv[c], in_=out_sb[:])

    nc.finalize()
    return nc


_CACHE = {}


def kernel(output, edge_index, edge_type, lm_w, lm_b, ln_g, ln_b,
           rgcn_root, rgcn_rel, rgcn_bias, wq, bq, wk, bk, wv, bv,
           wo, bo, bn_g, bn_b, cls_w, cls_b):
    global LAST_RESULT
    _ensure_profile_hook()

    output = np.asarray(output, np.float32)
    src = np.asarray(edge_index[0]).astype(np.int64)
    dst = np.asarray(edge_index[1]).astype(np.int64)
    et = np.asarray(edge_type).astype(np.int64)
    bq = np.asarray(bq, np.float32)
    if np.abs(bq).max() > 0:
        raise NotImplementedError("nonzero bq not supported by this kernel")

    # ---- host-side layout prep (index math only) ----
    outT = np.ascontiguousarray(output.reshape(N, D).T).astype(np.float16)  # [D, N]
    cnt = np.zeros((N, NR), np.float32)
    np.add.at(cnt, (dst, et), 1.0)
    scale_e = (1.0 / np.maximum(cnt, 1.0))[dst, et].astype(np.float32)
    # dense sparse-block adjacency per core: adj[p, c, r, d] = sum of
    # 1/max(cnt,1) over edges (src=c*128+p, type=r, dst=core_base+d)
    A = np.zeros((N, NR, N), np.float32)
    np.add.at(A, (src, et, dst), scale_e)
    A = A.reshape(16, 128, NR, NCORES, NPC).transpose(3, 1, 0, 2, 4)  # [core, p, c, r, d]
    per_core = [np.ascontiguousarray(A[c]).astype(np.float16) for c in range(NCORES)]
    EPAD = 0

    wo_pad = np.zeros((DH + 1, NH, HID), np.float32)
    for h in range(NH):
        wo_pad[1:, h, :] = wo[DH * h:DH * (h + 1), :]
    bo_eff = (np.asarray(bo, np.float64) + np.asarray(bv, np.float64) @ np.asarray(wo, np.float64)).astype(np.float32)
    rel_cat = np.concatenate([rgcn_rel[:, r, :, :] for r in range(NR)], axis=2)  # [NL, HID, NR*HID]

    shared = {
        "lm_w": np.asarray(lm_w, np.float16),
        "lm_b": np.asarray(lm_b, np.float32),
        "ln_g": np.asarray(ln_g, np.float32),
        "ln_b": np.asarray(ln_b, np.float32),
        "root": np.asarray(rgcn_root, np.float16),
        "rel": np.ascontiguousarray(rel_cat).astype(np.float16),
        "rgb": np.asarray(rgcn_bias, np.float32),
        "wq": np.asarray(wq, np.float16),
        "wk": np.asarray(wk, np.float16),
        "wv": np.asarray(wv, np.float16),
        "wop": wo_pad.astype(np.float16),
        "boe": bo_eff,
        "bn_g": np.asarray(bn_g, np.float32),
        "bn_b": np.asarray(bn_b, np.float32),
        "cls_w": np.asarray(cls_w, np.float16),
        "cls_b": np.asarray(cls_b, np.float32),
    }

    in_maps = []
    for c in range(NCORES):
        m = dict(shared)
        m["outT"] = np.ascontiguousarray(outT[:, c * NPC:(c + 1) * NPC])
        m["adj"] = per_core[c]
        in_maps.append(m)

    if EPAD not in _CACHE:
        nc = bacc.Bacc("TRN2")
        nc.num_devices = NCORES
        _CACHE[EPAD] = build(nc, EPAD)
    nc = _CACHE[EPAD]

    res = run_bass_kernel_spmd(nc, in_maps, core_ids=list(range(NCORES)))
    LAST_RESULT = res
    y = np.concatenate([res.results[c]["y"] for c in range(NCORES)], axis=0)
    return y.reshape(1, N, NH).astype(np.float32)


# revision 12
# speedup vs baseline: 1.0246x; 1.0246x over previous
"""Trainium2 Bass kernel for nn_CrossModelAttention (gnn_message_passing).

Distribution (8 NeuronCores, one SPMD NEFF):
  - lm head + LayerNorm: node-sharded (256 nodes/core), AllGather -> full lm^T
  - RGCN x2: relation GEMMs replicated (cheap), edge aggregation sharded by
    dst node (edges sorted by dst on host, per-core index/scale arrays);
    segment-sum done on the PE via per-tile one-hot selection matmuls;
    messages gathered with dma_gather from a per-core fp16 xr table.
    AllGather -> full g^T after each layer.
  - Attention: query-sharded (each core does all 8 heads for its 256 query
    nodes); no collective needed for the head merge.
  - Residual + BatchNorm: channel stats partial-summed locally, 1KB
    AllReduce, normalization + classifier local; per-core [256, 8] outputs
    concatenated on host.

Layouts: activations kept feature-major ("T layout", [128 feat partitions,
nodes free]) so per-feature params are per-partition scalars and no
transposes are needed anywhere. Matmul inputs fp16, PSUM/stats fp32.
"""

import os
import sys

if "/opt/trn_rl_repo" not in sys.path:
    sys.path.insert(0, "/opt/trn_rl_repo")

import numpy as np

import concourse.bacc as bacc
import concourse.bass as bass
import concourse.mybir as mybir
import concourse.tile as tile
from concourse.bass_utils import run_bass_kernel_spmd

F32 = mybir.dt.float32
F16 = mybir.dt.float16
I16 = mybir.dt.int16
AF = mybir.ActivationFunctionType
OP = mybir.AluOpType

N = 2048          # nodes (B*S)
D = 1024          # input dim
HID = 128
NR = 3            # relations
NL = 2            # rgcn layers
NH = 8            # heads
DH = 16
NCORES = 8
NPC = N // NCORES  # nodes per core = 256
EPS = 1e-5

LAST_RESULT = None  # BassKernelResults of the most recent run (for test harness)


def _ensure_profile_hook():
    """Install the NTFF profile hook if boot() could not (antenv.axon_hooks
    may be missing from the image). Only matters when BASS_TRACE=1."""
    try:
        import antenv.axon_hooks as ah
        if ah.get_axon_ntff_profile_hook() is None:
            from trn_agent_boot.trn_boot import _ntff_profile_via_ctypes
            hook = _ntff_profile_via_ctypes("/opt/axon/libaxon_pjrt.so")
            if hook is not None:
                ah.set_axon_ntff_profile_hook(hook)
    except Exception:
        pass


def _bcast_ap(dram_ap, parts, free):
    """DMA access pattern broadcasting a [free] dram vector across partitions."""
    return bass.AP(tensor=dram_ap.tensor, offset=dram_ap.offset, ap=[[0, parts], [1, free]])


def _gather_cc_ap(cc):
    """AP over cc_out [R, 128, NPC] reading as [128 feat, R*NPC nodes]."""
    return bass.AP(tensor=cc[:].tensor, offset=0,
                   ap=[[NPC, 128], [128 * NPC, NCORES], [1, NPC]])


def build(nc, EPAD):
    NT = EPAD // 128
    groups = [list(range(NCORES))]

    # ---------------- dram tensors ----------------
    outT_d = nc.dram_tensor("outT", [D, NPC], F16, kind="ExternalInput")
    lmw_d = nc.dram_tensor("lm_w", [D, HID], F16, kind="ExternalInput")
    lmb_d = nc.dram_tensor("lm_b", [HID], F32, kind="ExternalInput")
    lng_d = nc.dram_tensor("ln_g", [HID], F32, kind="ExternalInput")
    lnb_d = nc.dram_tensor("ln_b", [HID], F32, kind="ExternalInput")
    root_d = nc.dram_tensor("root", [NL, HID, HID], F16, kind="ExternalInput")
    rel_d = nc.dram_tensor("rel", [NL, HID, NR * HID], F16, kind="ExternalInput")
    rgb_d = nc.dram_tensor("rgb", [NL, HID], F32, kind="ExternalInput")
    wq_d = nc.dram_tensor("wq", [HID, HID], F16, kind="ExternalInput")
    wk_d = nc.dram_tensor("wk", [HID, HID], F16, kind="ExternalInput")
    wv_d = nc.dram_tensor("wv", [HID, HID], F16, kind="ExternalInput")
    wop_d = nc.dram_tensor("wop", [DH + 1, NH, HID], F16, kind="ExternalInput")
    boe_d = nc.dram_tensor("boe", [HID], F32, kind="ExternalInput")
    bng_d = nc.dram_tensor("bn_g", [HID], F32, kind="ExternalInput")
    bnb_d = nc.dram_tensor("bn_b", [HID], F32, kind="ExternalInput")
    clsw_d = nc.dram_tensor("cls_w", [HID, NH], F16, kind="ExternalInput")
    clsb_d = nc.dram_tensor("cls_b", [NH], F32, kind="ExternalInput")
    adj_d = nc.dram_tensor("adj", [128, N // 128, NR, NPC], F16, kind="ExternalInput")

    y_d = nc.dram_tensor("y", [NPC, NH], F32, kind="ExternalOutput")

    cc_in = [nc.dram_tensor(f"cci{i}", [128, NPC], F16, kind="Internal") for i in range(3)]
    cc_out = [nc.dram_tensor(f"cco{i}", [NCORES, 128, NPC], F16, kind="Internal",
                             addr_space="Shared") for i in range(3)]
    bn_in = nc.dram_tensor("bni", [128, 2], F32, kind="Internal")
    bn_out = nc.dram_tensor("bno", [128, 2], F32, kind="Internal", addr_space="Shared")

    with tile.TileContext(nc) as tc:
        with tc.tile_pool(name="const", bufs=1) as cst, \
             tc.tile_pool(name="persist", bufs=1) as per, \
             tc.tile_pool(name="work", bufs=2) as wk, \
             tc.tile_pool(name="small", bufs=2) as sm, \
             tc.tile_pool(name="epool", bufs=4) as ep, \
             tc.tile_pool(name="msgp", bufs=1) as mp, \
             tc.tile_pool(name="psA", bufs=2, space="PSUM") as psA, \
             tc.tile_pool(name="psB", bufs=1, space="PSUM") as psB, \
             tc.tile_pool(name="psAcc", bufs=1, space="PSUM") as psAcc, \
             tc.tile_pool(name="psSt", bufs=4, space="PSUM") as psSt:

            # ---------------- constants to SBUF ----------------
            lmw_sb = cst.tile([128, D // 128, HID], F16)
            nc.sync.dma_start(out=lmw_sb[:], in_=lmw_d[:].rearrange("(k p) f -> p k f", p=128))
            outT_sb = cst.tile([128, D // 128, NPC], F16)
            nc.sync.dma_start(out=outT_sb[:], in_=outT_d[:].rearrange("(k p) n -> p k n", p=128))
            root_sb = cst.tile([128, NL, HID], F16)
            nc.sync.dma_start(out=root_sb[:], in_=root_d[:].rearrange("l k f -> k l f"))
            rel_sb = cst.tile([128, NL, NR * HID], F16)
            nc.sync.dma_start(out=rel_sb[:], in_=rel_d[:].rearrange("l k f -> k l f"))
            wq_sb = cst.tile([128, HID], F16)
            nc.sync.dma_start(out=wq_sb[:], in_=wq_d[:])
            wk_sb = cst.tile([128, HID], F16)
            nc.sync.dma_start(out=wk_sb[:], in_=wk_d[:])
            wv_sb = cst.tile([128, HID], F16)
            nc.sync.dma_start(out=wv_sb[:], in_=wv_d[:])
            wop_sb = cst.tile([DH + 1, NH, HID], F16)
            nc.sync.dma_start(out=wop_sb[:], in_=wop_d[:])
            cls_sb = cst.tile([128, NH], F16)
            nc.sync.dma_start(out=cls_sb[:], in_=clsw_d[:])

            def vec128(d):
                t = cst.tile([128, 1], F32, tag=f"v_{d.name}")
                nc.sync.dma_start(out=t[:], in_=d[:, None])
                return t
            lmb_sb = vec128(lmb_d)
            lng_sb = vec128(lng_d)
            lnb_sb = vec128(lnb_d)
            bng_sb = vec128(bng_d)
            bnb_sb = vec128(bnb_d)
            boe_sb = vec128(boe_d)
            rgb_sb = cst.tile([128, NL], F32)
            nc.sync.dma_start(out=rgb_sb[:], in_=rgb_d[:].rearrange("l f -> f l"))
            clsb_bc = cst.tile([128, NH], F32)
            nc.gpsimd.dma_start(out=clsb_bc[:], in_=_bcast_ap(clsb_d[:], 128, NH))
            adj_sb = cst.tile([128, N // 128, NR, NPC], F16)
            nc.sync.dma_start(out=adj_sb[:], in_=adj_d[:])

            ones_col = cst.tile([128, 1], F32)
            nc.vector.memset(ones_col[:], 1.0)
            ones_1x128 = cst.tile([1, 128], F32)
            nc.vector.memset(ones_1x128[:], 1.0)
            ones_1x17 = cst.tile([1, DH + 1], F32)
            nc.vector.memset(ones_1x17[:], 1.0)
            eps1 = cst.tile([1, 1], F32)
            nc.vector.memset(eps1[:], EPS)
            eps128 = cst.tile([128, 1], F32)
            nc.vector.memset(eps128[:], EPS)

            def ps_acc():
                return psAcc.tile([128, NPC], F32, tag="acc", name="acc")

            def ps_mm():
                return psA.tile([128, NR * HID], F32, tag="mm", name="mm")

            def ps_b():
                return psB.tile([128, 512], F32, tag="b", name="b")

            # ---------------- phase 1: lm head + LN (own nodes) ----------------
            lm_ps = ps_acc()
            for k in range(D // 128):
                nc.tensor.matmul(lm_ps[:], lhsT=lmw_sb[:, k, :], rhs=outT_sb[:, k, :],
                                 start=(k == 0), stop=(k == D // 128 - 1))
            r_sb = per.tile([128, NPC], F32, tag="r")
            nc.scalar.activation(out=r_sb[:], in_=lm_ps[:], func=AF.Relu, bias=lmb_sb[:], scale=1.0)
            sq = wk.tile([128, NPC], F32, tag="sq")
            nc.vector.tensor_mul(sq[:], r_sb[:], r_sb[:])
            st_ps = ps_b()[0:1, :]
            nc.tensor.matmul(st_ps[:, 0:NPC], lhsT=ones_col[:], rhs=r_sb[:], start=True, stop=True)
            nc.tensor.matmul(st_ps[:, NPC:], lhsT=ones_col[:], rhs=sq[:], start=True, stop=True)
            mu_r = sm.tile([1, NPC], F32, tag="mu")
            nc.vector.tensor_scalar_mul(mu_r[:], st_ps[:, 0:NPC], 1.0 / HID)
            ex2_r = sm.tile([1, NPC], F32, tag="ex2")
            nc.vector.tensor_scalar_mul(ex2_r[:], st_ps[:, NPC:], 1.0 / HID)
            var_r = sm.tile([1, NPC], F32, tag="var")
            nc.vector.tensor_mul(var_r[:], mu_r[:], mu_r[:])
            nc.vector.tensor_sub(var_r[:], ex2_r[:], var_r[:])
            sd_r = sm.tile([1, NPC], F32, tag="sd")
            nc.scalar.activation(out=sd_r[:], in_=var_r[:], func=AF.Sqrt, bias=eps1[:], scale=1.0)
            pk = sm.tile([1, 2 * NPC], F32, tag="pk")
            nc.vector.reciprocal(pk[:, 0:NPC], sd_r[:])
            nc.vector.tensor_mul(pk[:, NPC:], mu_r[:], pk[:, 0:NPC])
            bc_ps = ps_b()
            nc.tensor.matmul(bc_ps[:], lhsT=ones_1x128[:], rhs=pk[:], start=True, stop=True)
            lmT_own = per.tile([128, NPC], F32, tag="lmT_own")
            nc.vector.tensor_mul(lmT_own[:], r_sb[:], bc_ps[:, 0:NPC])
            nc.vector.tensor_sub(lmT_own[:], lmT_own[:], bc_ps[:, NPC:])
            nc.vector.tensor_scalar(out=lmT_own[:], in0=lmT_own[:], scalar1=lng_sb[:],
                                    scalar2=lnb_sb[:], op0=OP.mult, op1=OP.add)
            lm16_own = per.tile([128, NPC], F16, tag="lm16_own")
            nc.vector.tensor_copy(lm16_own[:], lmT_own[:])

            q16_all = per.tile([DH, NH, NPC], F16, tag="q16a")
            for h in range(NH):
                q_ps = ps_b()[0:DH, 0:NPC]
                nc.tensor.matmul(q_ps[:], lhsT=wq_sb[:, DH * h:DH * (h + 1)], rhs=lm16_own[:],
                                 start=True, stop=True)
                nc.vector.tensor_copy(q16_all[:, h, :], q_ps[:])

            nc.sync.dma_start(out=cc_in[0][:], in_=lm16_own[:])
            nc.gpsimd.collective_compute(
                kind="AllGather", op=OP.bypass, replica_groups=groups,
                ins=[cc_in[0][:]], outs=[cc_out[0][:]])
            lmT_full = per.tile([128, N], F16, tag="lmT_full")
            nc.sync.dma_start(out=lmT_full[:].rearrange("f (r n) -> f r n", r=NCORES), in_=_gather_cc_ap(cc_out[0]))

            # ---------------- phase 2: RGCN layers ----------------
            xT = lmT_full
            x16_own = lm16_own
            g16_own = None
            for l in range(NL):
                xr_all = mp.tile([128, N // 128, NR, HID], F16, tag="xr_all", name="xr_all")
                for c in range(N // 128):
                    xr_ps = ps_mm()
                    nc.tensor.matmul(xr_ps[:], lhsT=xT[:, 128 * c:128 * (c + 1)],
                                     rhs=rel_sb[:, l, :], start=True, stop=True)
                    nc.vector.tensor_copy(
                        xr_all[:, c, :, :],
                        xr_ps[:].rearrange("p (r f) -> p r f", r=NR))
                agg_ps = ps_acc()
                nc.tensor.matmul(agg_ps[:], lhsT=root_sb[:, l, :], rhs=x16_own[:],
                                 start=True, stop=False)
                for c in range(N // 128):
                    for r in range(NR):
                        nc.tensor.matmul(agg_ps[:], lhsT=xr_all[:, c, r, :],
                                         rhs=adj_sb[:, c, r, :], start=False,
                                         stop=(c == N // 128 - 1 and r == NR - 1))
                g16_own = per.tile([128, NPC], F16, tag=f"g16_own{l}")
                nc.scalar.activation(out=g16_own[:], in_=agg_ps[:], func=AF.Relu,
                                     bias=rgb_sb[:, l:l + 1], scale=1.0)
                nc.sync.dma_start(out=cc_in[1 + l][:], in_=g16_own[:])
                nc.gpsimd.collective_compute(
                    kind="AllGather", op=OP.bypass, replica_groups=groups,
                    ins=[cc_in[1 + l][:]], outs=[cc_out[1 + l][:]])
                gT_full = per.tile([128, N], F16, tag=f"gT_full{l}")
                nc.sync.dma_start(out=gT_full[:].rearrange("f (r n) -> f r n", r=NCORES), in_=_gather_cc_ap(cc_out[1 + l]))
                xT = gT_full
                x16_own = g16_own

            gT = xT  # final graph features, feature-major, fp16

            # ---------------- phase 3: attention (all heads, own queries) ----------------
            vaug = per.tile([128, N // 128, NH, DH + 1], F16, tag="vaug")
            nc.vector.memset(vaug[:, :, :, 0:1], 1.0)
            for c in range(N // 128):
                v_ps = ps_mm()[:, 0:HID]
                nc.tensor.matmul(v_ps[:], lhsT=gT[:, 128 * c:128 * (c + 1)], rhs=wv_sb[:],
                                 start=True, stop=True)
                nc.vector.tensor_copy(
                    vaug[:, c, :, 1:DH + 1],
                    v_ps[:].rearrange("p (h d) -> p h d", h=NH))

            attn_ps = ps_acc()
            for h in range(NH):
                q16 = q16_all[:, h, :]
                k16 = wk.tile([DH, N], F16, tag="k16")
                for j in range(N // 512):
                    k_ps = ps_b()[0:DH, :]
                    nc.tensor.matmul(k_ps[:], lhsT=wk_sb[:, DH * h:DH * (h + 1)],
                                     rhs=gT[:, 512 * j:512 * (j + 1)], start=True, stop=True)
                    nc.vector.tensor_copy(k16[:, 512 * j:512 * (j + 1)], k_ps[:])
                num_ps = ps_b()[0:DH + 1, 0:NPC]
                for jp in range(N // 256):
                    st_ps = psSt.tile([128, 2, NPC], F32, tag="stps", name="stps")
                    for u in range(2):
                        j = 2 * jp + u
                        nc.tensor.matmul(st_ps[:, u, :], lhsT=k16[:, 128 * j:128 * (j + 1)],
                                         rhs=q16[:], start=True, stop=True)
                    e16 = ep.tile([128, 2, NPC], F16, tag="e16", name="e16")
                    nc.scalar.activation(out=e16[:], in_=st_ps[:], func=AF.Exp)
                    for u in range(2):
                        j = 2 * jp + u
                        nc.tensor.matmul(num_ps[:], lhsT=vaug[:, j, h, :], rhs=e16[:, u, :],
                                         start=(j == 0), stop=(j == N // 128 - 1))
                num_sb = sm.tile([DH + 1, NPC], F32, tag="num")
                nc.vector.tensor_copy(num_sb[:], num_ps[:])
                rden = sm.tile([1, NPC], F32, tag="rden")
                nc.vector.reciprocal(rden[:], num_sb[0:1, :])
                dbc_ps = ps_b()[0:DH + 1, 0:NPC]
                nc.tensor.matmul(dbc_ps[:], lhsT=ones_1x17[:], rhs=rden[:], start=True, stop=True)
                ctx16 = sm.tile([DH + 1, NPC], F16, tag="ctx16")
                nc.vector.tensor_mul(ctx16[:], num_sb[:], dbc_ps[:])
                nc.tensor.matmul(attn_ps[:], lhsT=wop_sb[:, h, :], rhs=ctx16[:],
                                 start=(h == 0), stop=(h == NH - 1))

            # ---------------- phase 4: residual + BN + classifier ----------------
            fused = per.tile([128, NPC], F32, tag="fused")
            nc.vector.scalar_tensor_tensor(out=fused[:], in0=attn_ps[:], scalar=boe_sb[:],
                                           in1=lmT_own[:], op0=OP.add, op1=OP.add)
            fsq = wk.tile([128, NPC], F32, tag="fsq")
            nc.vector.tensor_mul(fsq[:], fused[:], fused[:])
            bnp = sm.tile([128, 2], F32, tag="bnp")
            nc.vector.tensor_reduce(bnp[:, 0:1], fused[:], mybir.AxisListType.X, OP.add)
            nc.vector.tensor_reduce(bnp[:, 1:2], fsq[:], mybir.AxisListType.X, OP.add)
            nc.sync.dma_start(out=bn_in[:], in_=bnp[:])
            nc.gpsimd.collective_compute(
                kind="AllReduce", op=OP.add, replica_groups=groups,
                ins=[bn_in[:]], outs=[bn_out[:]])
            bnst = sm.tile([128, 2], F32, tag="bnst")
            nc.sync.dma_start(out=bnst[:], in_=bn_out[:])
            mu_c = sm.tile([128, 1], F32, tag="muc")
            nc.vector.tensor_scalar_mul(mu_c[:], bnst[:, 0:1], 1.0 / N)
            var_c = sm.tile([128, 1], F32, tag="varc")
            nc.vector.tensor_scalar_mul(var_c[:], bnst[:, 1:2], 1.0 / N)
            mu2_c = sm.tile([128, 1], F32, tag="mu2c")
            nc.vector.tensor_mul(mu2_c[:], mu_c[:], mu_c[:])
            nc.vector.tensor_sub(var_c[:], var_c[:], mu2_c[:])
            sd_c = sm.tile([128, 1], F32, tag="sdc")
            nc.scalar.activation(out=sd_c[:], in_=var_c[:], func=AF.Sqrt, bias=eps128[:], scale=1.0)
            scl_c = sm.tile([128, 1], F32, tag="sclc")
            nc.vector.reciprocal(scl_c[:], sd_c[:])
            nc.vector.tensor_mul(scl_c[:], scl_c[:], bng_sb[:])
            shf_c = sm.tile([128, 1], F32, tag="shfc")
            nc.vector.tensor_mul(shf_c[:], mu_c[:], scl_c[:])
            nc.vector.tensor_sub(shf_c[:], bnb_sb[:], shf_c[:])
            fn16 = wk.tile([128, NPC], F16, tag="fn16")
            nc.vector.tensor_scalar(out=fn16[:], in0=fused[:], scalar1=scl_c[:],
                                    scalar2=shf_c[:], op0=OP.mult, op1=OP.add)
            yv = y_d[:].rearrange("(c p) f -> c p f", p=128)
            for c in range(NPC // 128):
                lg_ps = ps_mm()[:, 0:NH]
                nc.tensor.matmul(lg_ps[:], lhsT=fn16[:, 128 * c:128 * (c + 1)], rhs=cls_sb[:],
                                 start=True, stop=True)
                out_sb = wk.tile([128, NH], F32, tag="outsb")
                nc.vector.tensor_add(out_sb[:], lg_ps[:], clsb_bc[:])
                nc.sync.dma_start(out=yv[c], in_=out_sb[:])

    nc.finalize()
    return nc


_CACHE = {}


def kernel(output, edge_index, edge_type, lm_w, lm_b, ln_g, ln_b,
           rgcn_root, rgcn_rel, rgcn_bias, wq, bq, wk, bk, wv, bv,
           wo, bo, bn_g, bn_b, cls_w, cls_b):
    global LAST_RESULT
    _ensure_profile_hook()

    output = np.asarray(output, np.float32)
    src = np.asarray(edge_index[0]).astype(np.int64)
    dst = np.asarray(edge_index[1]).astype(np.int64)
    et = np.asarray(edge_type).astype(np.int64)
    bq = np.asarray(bq, np.float32)
    if np.abs(bq).max() > 0:
        raise NotImplementedError("nonzero bq not supported by this kernel")

    # ---- host-side layout prep (index math only) ----
    outT = np.ascontiguousarray(output.reshape(N, D).T).astype(np.float16)  # [D, N]
    cnt = np.zeros((N, NR), np.float32)
    np.add.at(cnt, (dst, et), 1.0)
    scale_e = (1.0 / np.maximum(cnt, 1.0))[dst, et].astype(np.float32)
    # dense sparse-block adjacency per core: adj[p, c, r, d] = sum of
    # 1/max(cnt,1) over edges (src=c*128+p, type=r, dst=core_base+d)
    A = np.zeros((N, NR, N), np.float32)
    np.add.at(A, (src, et, dst), scale_e)
    A = A.reshape(16, 128, NR, NCORES, NPC).transpose(3, 1, 0, 2, 4)  # [core, p, c, r, d]
    per_core = [np.ascontiguousarray(A[c]).astype(np.float16) for c in range(NCORES)]
    EPAD = 0

    wo_pad = np.zeros((DH + 1, NH, HID), np.float32)
    for h in range(NH):
        wo_pad[1:, h, :] = wo[DH * h:DH * (h + 1), :]
    bo_eff = (np.asarray(bo, np.float64) + np.asarray(bv, np.float64) @ np.asarray(wo, np.float64)).astype(np.float32)
    rel_cat = np.concatenate([rgcn_rel[:, r, :, :] for r in range(NR)], axis=2)  # [NL, HID, NR*HID]

    shared = {
        "lm_w": np.asarray(lm_w, np.float16),
        "lm_b": np.asarray(lm_b, np.float32),
        "ln_g": np.asarray(ln_g, np.float32),
        "ln_b": np.asarray(ln_b, np.float32),
        "root": np.asarray(rgcn_root, np.float16),
        "rel": np.ascontiguousarray(rel_cat).astype(np.float16),
        "rgb": np.asarray(rgcn_bias, np.float32),
        "wq": np.asarray(wq, np.float16),
        "wk": np.asarray(wk, np.float16),
        "wv": np.asarray(wv, np.float16),
        "wop": wo_pad.astype(np.float16),
        "boe": bo_eff,
        "bn_g": np.asarray(bn_g, np.float32),
        "bn_b": np.asarray(bn_b, np.float32),
        "cls_w": np.asarray(cls_w, np.float16),
        "cls_b": np.asarray(cls_b, np.float32),
    }

    in_maps = []
    for c in range(NCORES):
        m = dict(shared)
        m["outT"] = np.ascontiguousarray(outT[:, c * NPC:(c + 1) * NPC])
        m["adj"] = per_core[c]
        in_maps.append(m)

    if EPAD not in _CACHE:
        nc = bacc.Bacc("TRN2")
        nc.num_devices = NCORES
        _CACHE[EPAD] = build(nc, EPAD)
    nc = _CACHE[EPAD]

    res = run_bass_kernel_spmd(nc, in_maps, core_ids=list(range(NCORES)))
    LAST_RESULT = res
    y = np.concatenate([res.results[c]["y"] for c in range(NCORES)], axis=0)
    return y.reshape(1, N, NH).astype(np.float32)
